# revision 17
# baseline (speedup 1.0000x reference)
"""Multi-head attention (B=4, S=2048, E=1024, H=16) on 8 NeuronCores.

Sharding: data-parallel over (batch, query-half): core c handles batch c//2,
query rows (c%2)*1024:(c%2+1)*1024, with the full K/V rows of that batch.
No collectives; output slices are disjoint and concatenated on host.
Activations are distributed to each core pre-transposed ([E, rows] layout).

Per-core program, PE-tiled attention (v2):
  PV-proj  V[k, d] per head via lhsT=vinT chunks, rhs=Wv (streamed once).
  PK/PQ    per head-pair p: kT_p/qT_p [128, S] with head 2p on partitions
           0:64 and head 2p+1 on 64:128 (compact, no zero padding).
  PA       per (pair, qc, kt): two K=64 S^T-matmuls row-tiled (tile_position
           (0,0)/(64,0)) run CONCURRENT on the PE; one [128, 2, 512] exp on
           ACT; two M=64 PV matmuls col-tiled ((0,0)/(0,64)) concurrent; two
           M=64 denominator ones-matmuls col-tiled, accumulated in PSUM
           across kt.  Softmax denominator therefore costs no DVE folds.
  finalize reciprocal_approx_fast(den) -> rb; ctx_sb = ctx_ps * rb (DVE).
  PO       out = ctx^T-tiles^T @ Wo + ones x (bv@Wo + bo).
Emission software-pipelines everything: V-proj eighths interleave with pair-0
attention; PK/PQ(p+1) chunks interleave with pair-p attention so the scalar
engine (exp) never starves and the PE never idles.
"""

import os
import sys

for _p in ("/opt/trn_rl_repo", os.path.expanduser("~/.axon_site/_ro/trn_rl_repo")):
    if os.path.isdir(_p) and _p not in sys.path:
        sys.path.append(_p)

import numpy as np

import concourse.bass as bass
import concourse.tile as tile
from concourse import bacc, mybir
from concourse.bass_utils import run_bass_kernel_spmd

E = 1024
H = 16
D = 64
B = 4
S = 2048
P = 128
RQ = 1024  # query rows per core
RK = 2048  # kv rows per core
F32 = mybir.dt.float32
BF16 = mybir.dt.bfloat16
N_CORES = 8

ET = E // P  # 8 e-tiles == 8 head pairs
NKT = RK // P  # 16 k-tiles
NQC = RQ // 512  # 2 q-chunks

_CACHE = {}
_LAST_RESULTS = None


def _build_program():
    nc = bacc.Bacc("TRN2", target_bir_lowering=False, debug=False, num_devices=N_CORES)

    qinT_d = nc.dram_tensor("qinT", [E, RQ], BF16, kind="ExternalInput").ap()
    kinT_d = nc.dram_tensor("kinT", [E, RK], BF16, kind="ExternalInput").ap()
    vinT_d = nc.dram_tensor("vinT", [E, RK], BF16, kind="ExternalInput").ap()
    Wq = nc.dram_tensor("Wq", [E, E], BF16, kind="ExternalInput").ap()
    Wk = nc.dram_tensor("Wk", [E, E], BF16, kind="ExternalInput").ap()
    Wv = nc.dram_tensor("Wv", [E, E], BF16, kind="ExternalInput").ap()
    Wo = nc.dram_tensor("Wo", [E, E], BF16, kind="ExternalInput").ap()
    bq = nc.dram_tensor("bq", [E], F32, kind="ExternalInput").ap()
    bk = nc.dram_tensor("bk", [E], F32, kind="ExternalInput").ap()
    bv = nc.dram_tensor("bv", [E], F32, kind="ExternalInput").ap()
    bo = nc.dram_tensor("bo", [E], F32, kind="ExternalInput").ap()
    row_d = nc.dram_tensor("row", [E], F32, kind="ExternalInput").ap()
    out = nc.dram_tensor("out", [RQ, E], F32, kind="ExternalOutput").ap()

    with tile.TileContext(nc) as tc:
        with (
            tc.tile_pool(name="const", bufs=1) as const,
            tc.tile_pool(name="persist", bufs=1) as persist,
            tc.tile_pool(name="kin", bufs=1) as kin_pool,
            tc.tile_pool(name="qin", bufs=1) as qin_pool,
            tc.tile_pool(name="kt", bufs=2) as kT_pool,
            tc.tile_pool(name="qt", bufs=2) as qT_pool,
            tc.tile_pool(name="exp", bufs=4) as exp_pool,
            tc.tile_pool(name="rb", bufs=2) as rb_pool,
            tc.tile_pool(name="ps_s", bufs=2, space="PSUM") as s_psum,
            tc.tile_pool(name="ps_ctx", bufs=1, space="PSUM") as ctx_psum,
            tc.tile_pool(name="ps_den", bufs=1, space="PSUM") as den_psum,
            tc.tile_pool(name="ps_proj", bufs=2, space="PSUM") as proj_psum,
        ):
            # ---- weights needed first (wk gates the very first matmul) ----
            wo_cm = tc.tile_pool(name="w_o", bufs=1)
            wo_pool = wo_cm.__enter__()
            wo_sb = wo_pool.tile([P, ET, E], BF16)
            w_cm = tc.tile_pool(name="w_kqv", bufs=1)
            w_pool = w_cm.__enter__()
            wv_sb = w_pool.tile([P, ET, E], BF16)
            wk_sb = w_pool.tile([P, ET, E], BF16)
            wq_sb = w_pool.tile([P, ET, E], BF16)
            for ke in range(ET):
                nc.sync.dma_start(out=wk_sb[:, ke, :], in_=Wk[ke * P : (ke + 1) * P, :])

            # ---- constants -------------------------------------------------
            bq_sb = const.tile([P, ET], F32)
            nc.sync.dma_start(out=bq_sb[:], in_=bq.rearrange("(t p) -> p t", p=P))
            bk_sb = const.tile([P, ET], F32)
            nc.sync.dma_start(out=bk_sb[:], in_=bk.rearrange("(t p) -> p t", p=P))
            row_f32 = const.tile([1, E], F32)
            nc.sync.dma_start(out=row_f32[:], in_=row_d.rearrange("(p e) -> p e", p=1))
            row_pad = const.tile([P, E], BF16)
            nc.gpsimd.memset(row_pad[:], 0.0)
            nc.vector.tensor_copy(row_pad[0:1, :], row_f32[:])
            ones_scr = const.tile([P, P], F32)
            nc.vector.memset(ones_scr[:], 1.0)
            ones_mat = const.tile([P, P], BF16)
            nc.vector.tensor_copy(ones_mat[:], ones_scr[:])

            # ---- resident inputs (chunked so PK(0) can start early) -------
            kin_sb = kin_pool.tile([P, ET, RK], BF16)  # 32KB/part
            for c in range(4):
                for ke in range(ET):
                    nc.sync.dma_start(
                        out=kin_sb[:, ke, c * 512 : (c + 1) * 512],
                        in_=kinT_d[ke * P : (ke + 1) * P, c * 512 : (c + 1) * 512],
                    )
            qin_sb = qin_pool.tile([P, ET, RQ], BF16)  # 16KB/part
            for c in range(2):
                for ke in range(ET):
                    nc.sync.dma_start(
                        out=qin_sb[:, ke, c * 512 : (c + 1) * 512],
                        in_=qinT_d[ke * P : (ke + 1) * P, c * 512 : (c + 1) * 512],
                    )

            # persistent state
            v_sb = persist.tile([P, NKT, H, D], BF16)  # 32KB/part
            ctx_sb = persist.tile([P, ET, RQ], BF16)  # 16KB/part

            # remaining weights (after kin/qin so they don't delay PK(0))
            for ke in range(ET):
                nc.sync.dma_start(out=wq_sb[:, ke, :], in_=Wq[ke * P : (ke + 1) * P, :])
            for ke in range(ET):
                nc.sync.dma_start(out=wv_sb[:, ke, :], in_=Wv[ke * P : (ke + 1) * P, :])

            kT = {}  # pair -> [P, RK] tile, head 2p on parts 0:64, 2p+1 on 64:128
            qT = {}  # pair -> [P, RQ] tile

            # ---- emission helpers -----------------------------------------
            proj_q = []  # closures, one half-chunk (4 matmuls) each
            _chunk_state = {}

            def _emit_half(w_sb, in_sb, dst, bias, p, c, half):
                """Half of one projection chunk; halves share one psum tile."""
                if half == 0:
                    _chunk_state[(id(dst), c)] = proj_psum.tile(
                        [P, 512], F32, tag="proj", name="proj_ps"
                    )
                ps = _chunk_state[(id(dst), c)]
                for ke in range(4 * half, 4 * half + 4):
                    nc.tensor.matmul(
                        ps[:],
                        lhsT=w_sb[:, ke, p * P : (p + 1) * P],
                        rhs=in_sb[:, ke, c * 512 : (c + 1) * 512],
                        start=(ke == 0),
                        stop=(ke == ET - 1),
                    )
                if half == 1:
                    nc.vector.tensor_scalar_add(
                        dst[:, c * 512 : (c + 1) * 512], ps[:], bias
                    )
                    del _chunk_state[(id(dst), c)]

            def emit_pk_chunk(p, c, half=None):
                if c == 0 and (half is None or half == 0):
                    kT[p] = kT_pool.tile([P, RK], BF16, tag="kt", name=f"kT{p}")
                for h_ in ((0, 1) if half is None else (half,)):
                    _emit_half(wk_sb, kin_sb, kT[p], bk_sb[:, p : p + 1], p, c, h_)

            def emit_pq_chunk(p, c, half=None):
                if c == 0 and (half is None or half == 0):
                    qT[p] = qT_pool.tile([P, RQ], BF16, tag="qt", name=f"qT{p}")
                for h_ in ((0, 1) if half is None else (half,)):
                    _emit_half(wq_sb, qin_sb, qT[p], bq_sb[:, p : p + 1], p, c, h_)

            vin_cm = tc.tile_pool(name="vin", bufs=2)
            vin_pool = vin_cm.__enter__()

            def emit_v_eighth(j):
                """Project V for k-rows [j*256, (j+1)*256), all 16 heads."""
                k0 = j * 256
                vin_t = vin_pool.tile([P, ET, 256], BF16, tag="vin")
                for ke in range(ET):
                    nc.sync.dma_start(
                        out=vin_t[:, ke, :],
                        in_=vinT_d[ke * P : (ke + 1) * P, k0 : k0 + 256],
                    )
                for kt_ in range(2):  # two 128-k-tiles per eighth
                    ktg = 2 * j + kt_
                    for ch in range(2):  # 512 e-cols = 8 heads each
                        ps = proj_psum.tile([P, 512], F32, tag="proj")
                        for ke in range(ET):
                            nc.tensor.matmul(
                                ps[:],
                                lhsT=vin_t[:, ke, kt_ * P : (kt_ + 1) * P],
                                rhs=wv_sb[:, ke, ch * 512 : (ch + 1) * 512],
                                start=(ke == 0),
                                stop=(ke == ET - 1),
                            )
                        nc.vector.tensor_copy(
                            v_sb[:, ktg, ch * 8 : (ch + 1) * 8, :], ps[:]
                        )

            # ---- attention -------------------------------------------------
            state = {"pend": None, "ctx": None, "den": None}

            def emit_pv_den(p, exp_t, kt_):
                """PV + denominator matmuls for one (pair, kt) step."""
                ctx_ps, den_ps = state["ctx"], state["den"]
                for h in range(2):
                    nc.tensor.matmul(
                        ctx_ps[h * D : (h + 1) * D, :],
                        lhsT=v_sb[:, kt_, 2 * p + h, :],
                        rhs=exp_t[:, h, :],
                        start=(kt_ == 0),
                        stop=(kt_ == NKT - 1),
                    )
                for h in range(2):
                    nc.tensor.matmul(
                        den_ps[h * D : (h + 1) * D, :],
                        lhsT=ones_mat[:, 0:D],
                        rhs=exp_t[:, h, :],
                        start=(kt_ == 0),
                        stop=(kt_ == NKT - 1),
                    )

            def attn_group(p, qc, kt_):
                """S + exp for (p, qc, kt); PV/den for the previous step."""
                if kt_ == 0:
                    state["ctx"] = ctx_psum.tile([P, 512], F32, tag="ctx", name="ctx_ps")
                    state["den"] = den_psum.tile([P, 512], F32, tag="den", name="den_ps")
                s = s_psum.tile([P, 2, 512], F32, tag="s")
                for h in range(2):
                    nc.tensor.matmul(
                        s[:, h, :],
                        lhsT=kT[p][h * D : (h + 1) * D, kt_ * P : (kt_ + 1) * P],
                        rhs=qT[p][h * D : (h + 1) * D, qc * 512 : (qc + 1) * 512],
                        start=True,
                        stop=True,
                    )
                exp_t = exp_pool.tile([P, 2, 512], BF16, tag="exp")
                nc.scalar.activation(
                    exp_t[:], s[:], mybir.ActivationFunctionType.Exp, scale=0.125
                )
                if state["pend"] is not None:
                    emit_pv_den(*state["pend"])
                state["pend"] = (p, exp_t, kt_)

            def finalize(p, qc):
                emit_pv_den(*state["pend"])
                state["pend"] = None
                ctx_ps, den_ps = state["ctx"], state["den"]
                rb = rb_pool.tile([P, 512], F32, tag="rb")
                nc.vector.reciprocal_approx_fast(out=rb[:], in_=den_ps[:])
                nc.vector.tensor_mul(
                    ctx_sb[:, p, qc * 512 : (qc + 1) * 512], ctx_ps[:], rb[:]
                )

            # ---- schedule --------------------------------------------------
            # prologue: K/Q projections for pair 0
            for c in range(4):
                emit_pk_chunk(0, c)
            for c in range(2):
                emit_pq_chunk(0, c)

            # pair 0, qc 0: V-projection eighth j feeds k-tiles 2j, 2j+1;
            # PK(1) chunks 0-2 ride along late in the sweep.
            for j in range(8):
                emit_v_eighth(j)
                attn_group(0, 0, 2 * j)
                attn_group(0, 0, 2 * j + 1)
                if j >= 5:
                    emit_pk_chunk(1, j - 5)
            finalize(0, 0)

            # remaining (pair, qc) steps with interleaved next-pair projections
            for p in range(ET):
                for qc in range(NQC):
                    if p == 0 and qc == 0:
                        continue
                    # spread the 6 projection chunks of pair p+1 over qc 0/1
                    tasks = []
                    if p + 1 < ET:
                        if qc == 0:
                            tasks = [
                                (emit_pk_chunk, p + 1, 0),
                                (emit_pk_chunk, p + 1, 1),
                                (emit_pk_chunk, p + 1, 2),
                            ]
                        else:
                            tasks = [
                                (emit_pk_chunk, p + 1, 3),
                                (emit_pq_chunk, p + 1, 0),
                                (emit_pq_chunk, p + 1, 1),
                            ]
                    for kt_ in range(NKT):
                        attn_group(p, qc, kt_)
                        if kt_ % 5 == 2 and tasks:
                            fn, a0, a1 = tasks.pop(0)
                            fn(a0, a1)
                    for fn, a0, a1 in tasks:
                        fn(a0, a1)
                    if p == 6 and qc == 1:
                        for ke in range(ET):
                            nc.sync.dma_start(
                                out=wo_sb[:, ke, :], in_=Wo[ke * P : (ke + 1) * P, :]
                            )
                    finalize(p, qc)

            vin_cm.__exit__(None, None, None)
            w_cm.__exit__(None, None, None)

            # ---- PO: out = ctx^T @ Wo + ones x (bv@Wo + bo) ----------------
            with tc.tile_pool(name="po_out", bufs=4) as out_pool:
                for qt in range(RQ // P):
                    for ch in range(2):
                        ps = proj_psum.tile([P, 512], F32, tag="proj")
                        for ke in range(ET):
                            nc.tensor.matmul(
                                ps[:],
                                lhsT=ctx_sb[:, ke, qt * P : (qt + 1) * P],
                                rhs=wo_sb[:, ke, ch * 512 : (ch + 1) * 512],
                                start=(ke == 0),
                                stop=False,
                            )
                        nc.tensor.matmul(
                            ps[:],
                            lhsT=ones_mat[:],
                            rhs=row_pad[:, ch * 512 : (ch + 1) * 512],
                            start=False,
                            stop=True,
                        )
                        out_t = out_pool.tile([P, 512], F32, tag="out_t")
                        if (qt + ch) % 2 == 0:
                            nc.vector.tensor_copy(out_t[:], ps[:])
                        else:
                            nc.scalar.copy(out_t[:], ps[:])
                        nc.sync.dma_start(
                            out=out[qt * P : (qt + 1) * P, ch * 512 : (ch + 1) * 512],
                            in_=out_t[:],
                        )

            wo_cm.__exit__(None, None, None)

    nc.compile()
    return nc


def _get_program():
    if "nc" not in _CACHE:
        _CACHE["nc"] = _build_program()
    return _CACHE["nc"]


def kernel(query, key, value, Wq, Wk, Wv, Wo, bq, bk, bv, bo):
    global _LAST_RESULTS
    query = np.asarray(query, dtype=np.float32)
    key = np.asarray(key, dtype=np.float32)
    value = np.asarray(value, dtype=np.float32)
    import ml_dtypes

    bf16 = ml_dtypes.bfloat16
    shared = {
        "Wq": np.ascontiguousarray(np.asarray(Wq, np.float32).astype(bf16)),
        "Wk": np.ascontiguousarray(np.asarray(Wk, np.float32).astype(bf16)),
        "Wv": np.ascontiguousarray(np.asarray(Wv, np.float32).astype(bf16)),
        "Wo": np.ascontiguousarray(np.asarray(Wo, np.float32).astype(bf16)),
        "bq": np.ascontiguousarray(np.asarray(bq, np.float32)),
        "bk": np.ascontiguousarray(np.asarray(bk, np.float32)),
        "bv": np.ascontiguousarray(np.asarray(bv, np.float32)),
        "bo": np.ascontiguousarray(np.asarray(bo, np.float32)),
        "row": np.ascontiguousarray(
            (np.asarray(bv, np.float64) @ np.asarray(Wo, np.float64)
             + np.asarray(bo, np.float64)).astype(np.float32)
        ),
    }
    in_maps = []
    for c in range(N_CORES):
        b, half = c // 2, c % 2
        in_maps.append(
            {
                "qinT": np.ascontiguousarray(
                    query[b, half * RQ : (half + 1) * RQ, :].T.astype(bf16)
                ),
                "kinT": np.ascontiguousarray(key[b].T.astype(bf16)),
                "vinT": np.ascontiguousarray(value[b].T.astype(bf16)),
                **shared,
            }
        )
    nc = _get_program()
    res = run_bass_kernel_spmd(nc, in_maps, list(range(N_CORES)))
    _LAST_RESULTS = res
    full = np.empty((B, S, E), dtype=np.float32)
    for c in range(N_CORES):
        b, half = c // 2, c % 2
        full[b, half * RQ : (half + 1) * RQ, :] = res.results[c]["out"]
    return full


# revision 20
# speedup vs baseline: 1.0741x; 1.0741x over previous
"""Multi-head attention (B=4, S=2048, E=1024, H=16) on 8 NeuronCores.

Sharding: data-parallel over (batch, query-half): core c handles batch c//2,
query rows (c%2)*1024:(c%2+1)*1024, with the full K/V rows of that batch.
No collectives; output slices are disjoint and concatenated on host.
Activations are distributed to each core pre-transposed ([E, rows] layout).

Per-core program, PE-tiled attention (v2):
  PV-proj  V[k, d] per head via lhsT=vinT chunks, rhs=Wv (streamed once).
  PK/PQ    per head-pair p: kT_p/qT_p [128, S] with head 2p on partitions
           0:64 and head 2p+1 on 64:128 (compact, no zero padding).
  PA       per (pair, qc, kt): two K=64 S^T-matmuls row-tiled (tile_position
           (0,0)/(64,0)) run CONCURRENT on the PE; one [128, 2, 512] exp on
           ACT; two M=64 PV matmuls col-tiled ((0,0)/(0,64)) concurrent; two
           M=64 denominator ones-matmuls col-tiled, accumulated in PSUM
           across kt.  Softmax denominator therefore costs no DVE folds.
  finalize reciprocal_approx_fast(den) -> rb; ctx_sb = ctx_ps * rb (DVE).
  PO       out = ctx^T-tiles^T @ Wo + ones x (bv@Wo + bo).
Emission software-pipelines everything: V-proj eighths interleave with pair-0
attention; PK/PQ(p+1) chunks interleave with pair-p attention so the scalar
engine (exp) never starves and the PE never idles.
"""

import os
import sys

for _p in ("/opt/trn_rl_repo", os.path.expanduser("~/.axon_site/_ro/trn_rl_repo")):
    if os.path.isdir(_p) and _p not in sys.path:
        sys.path.append(_p)

import numpy as np

import concourse.bass as bass
import concourse.tile as tile
from concourse import bacc, mybir
from concourse.bass_utils import run_bass_kernel_spmd

E = 1024
H = 16
D = 64
B = 4
S = 2048
P = 128
RQ = 1024  # query rows per core
RK = 2048  # kv rows per core
F32 = mybir.dt.float32
BF16 = mybir.dt.bfloat16
N_CORES = 8

ET = E // P  # 8 e-tiles == 8 head pairs
NKT = RK // P  # 16 k-tiles
NQC = RQ // 512  # 2 q-chunks

_CACHE = {}
_LAST_RESULTS = None


def _build_program():
    nc = bacc.Bacc("TRN2", target_bir_lowering=False, debug=False, num_devices=N_CORES)

    qinT_d = nc.dram_tensor("qinT", [E, RQ], BF16, kind="ExternalInput").ap()
    kinT_d = nc.dram_tensor("kinT", [E, RK], BF16, kind="ExternalInput").ap()
    vinT_d = nc.dram_tensor("vinT", [E, RK], BF16, kind="ExternalInput").ap()
    Wq = nc.dram_tensor("Wq", [E, E], BF16, kind="ExternalInput").ap()
    Wk = nc.dram_tensor("Wk", [E, E], BF16, kind="ExternalInput").ap()
    Wv = nc.dram_tensor("Wv", [E, E], BF16, kind="ExternalInput").ap()
    Wo = nc.dram_tensor("Wo", [E, E], BF16, kind="ExternalInput").ap()
    bq = nc.dram_tensor("bq", [E], F32, kind="ExternalInput").ap()
    bk = nc.dram_tensor("bk", [E], F32, kind="ExternalInput").ap()
    bv = nc.dram_tensor("bv", [E], F32, kind="ExternalInput").ap()
    bo = nc.dram_tensor("bo", [E], F32, kind="ExternalInput").ap()
    row_d = nc.dram_tensor("row", [E], F32, kind="ExternalInput").ap()
    out = nc.dram_tensor("out", [RQ, E], F32, kind="ExternalOutput").ap()

    with tile.TileContext(nc) as tc:
        with (
            tc.tile_pool(name="const", bufs=1) as const,
            tc.tile_pool(name="persist", bufs=1) as persist,
            tc.tile_pool(name="kin", bufs=1) as kin_pool,
            tc.tile_pool(name="qin", bufs=1) as qin_pool,
            tc.tile_pool(name="kt", bufs=2) as kT_pool,
            tc.tile_pool(name="qt", bufs=2) as qT_pool,
            tc.tile_pool(name="exp", bufs=4) as exp_pool,
            tc.tile_pool(name="rb", bufs=2) as rb_pool,
            tc.tile_pool(name="ps_s", bufs=2, space="PSUM") as s_psum,
            tc.tile_pool(name="ps_ctx", bufs=1, space="PSUM") as ctx_psum,
            tc.tile_pool(name="ps_den", bufs=1, space="PSUM") as den_psum,
            tc.tile_pool(name="ps_proj", bufs=2, space="PSUM") as proj_psum,
        ):
            # ---- weights needed first (wk gates the very first matmul) ----
            wo_cm = tc.tile_pool(name="w_o", bufs=1)
            wo_pool = wo_cm.__enter__()
            wo_sb = wo_pool.tile([P, ET, E], BF16)
            w_cm = tc.tile_pool(name="w_kqv", bufs=1)
            w_pool = w_cm.__enter__()
            wv_sb = w_pool.tile([P, ET, E], BF16)
            wk_sb = w_pool.tile([P, ET, E], BF16)
            wq_sb = w_pool.tile([P, ET, E], BF16)
            for ke in range(ET):
                nc.sync.dma_start(out=wk_sb[:, ke, :], in_=Wk[ke * P : (ke + 1) * P, :])

            # ---- constants -------------------------------------------------
            bq_sb = const.tile([P, ET], F32)
            nc.sync.dma_start(out=bq_sb[:], in_=bq.rearrange("(t p) -> p t", p=P))
            bk_sb = const.tile([P, ET], F32)
            nc.sync.dma_start(out=bk_sb[:], in_=bk.rearrange("(t p) -> p t", p=P))
            row_f32 = const.tile([1, E], F32)
            nc.sync.dma_start(out=row_f32[:], in_=row_d.rearrange("(p e) -> p e", p=1))
            row_pad = const.tile([P, E], BF16)
            nc.gpsimd.memset(row_pad[:], 0.0)
            nc.vector.tensor_copy(row_pad[0:1, :], row_f32[:])
            ones_scr = const.tile([P, P], F32)
            nc.vector.memset(ones_scr[:], 1.0)
            ones_mat = const.tile([P, P], BF16)
            nc.vector.tensor_copy(ones_mat[:], ones_scr[:])

            # ---- resident inputs: critical-path first -----------------------
            # order: kin chunk0 + qin chunk0 + wq unblock PK(0,c0)/PQ(0,c0),
            # then the rest streams in behind.
            kin_sb = kin_pool.tile([P, ET, RK], BF16)  # 32KB/part
            qin_sb = qin_pool.tile([P, ET, RQ], BF16)  # 16KB/part
            for ke in range(ET):
                nc.sync.dma_start(
                    out=kin_sb[:, ke, 0:512], in_=kinT_d[ke * P : (ke + 1) * P, 0:512]
                )
            for ke in range(ET):
                nc.sync.dma_start(
                    out=qin_sb[:, ke, 0:512], in_=qinT_d[ke * P : (ke + 1) * P, 0:512]
                )
            for ke in range(ET):
                nc.sync.dma_start(out=wq_sb[:, ke, :], in_=Wq[ke * P : (ke + 1) * P, :])
            for c in range(1, 4):
                for ke in range(ET):
                    nc.sync.dma_start(
                        out=kin_sb[:, ke, c * 512 : (c + 1) * 512],
                        in_=kinT_d[ke * P : (ke + 1) * P, c * 512 : (c + 1) * 512],
                    )
            for ke in range(ET):
                nc.sync.dma_start(
                    out=qin_sb[:, ke, 512:1024],
                    in_=qinT_d[ke * P : (ke + 1) * P, 512:1024],
                )

            # persistent state
            v_sb = persist.tile([P, NKT, H, D], BF16)  # 32KB/part
            ctx_sb = persist.tile([P, ET, RQ], BF16)  # 16KB/part

            # wv last (V-projection starts after PK/PQ(0) anyway)
            for ke in range(ET):
                nc.sync.dma_start(out=wv_sb[:, ke, :], in_=Wv[ke * P : (ke + 1) * P, :])

            kT = {}  # pair -> [P, RK] tile, head 2p on parts 0:64, 2p+1 on 64:128
            qT = {}  # pair -> [P, RQ] tile

            # ---- emission helpers -----------------------------------------
            proj_q = []  # closures, one half-chunk (4 matmuls) each
            _chunk_state = {}

            def _emit_half(w_sb, in_sb, dst, bias, p, c, half):
                """Half of one projection chunk; halves share one psum tile."""
                if half == 0:
                    _chunk_state[(id(dst), c)] = proj_psum.tile(
                        [P, 512], F32, tag="proj", name="proj_ps"
                    )
                ps = _chunk_state[(id(dst), c)]
                for ke in range(4 * half, 4 * half + 4):
                    nc.tensor.matmul(
                        ps[:],
                        lhsT=w_sb[:, ke, p * P : (p + 1) * P],
                        rhs=in_sb[:, ke, c * 512 : (c + 1) * 512],
                        start=(ke == 0),
                        stop=(ke == ET - 1),
                    )
                if half == 1:
                    nc.vector.tensor_scalar_add(
                        dst[:, c * 512 : (c + 1) * 512], ps[:], bias
                    )
                    del _chunk_state[(id(dst), c)]

            def emit_pk_chunk(p, c, half=None):
                if c == 0 and (half is None or half == 0):
                    kT[p] = kT_pool.tile([P, RK], BF16, tag="kt", name=f"kT{p}")
                for h_ in ((0, 1) if half is None else (half,)):
                    _emit_half(wk_sb, kin_sb, kT[p], bk_sb[:, p : p + 1], p, c, h_)

            def emit_pq_chunk(p, c, half=None):
                if c == 0 and (half is None or half == 0):
                    qT[p] = qT_pool.tile([P, RQ], BF16, tag="qt", name=f"qT{p}")
                for h_ in ((0, 1) if half is None else (half,)):
                    _emit_half(wq_sb, qin_sb, qT[p], bq_sb[:, p : p + 1], p, c, h_)

            vin_cm = tc.tile_pool(name="vin", bufs=2)
            vin_pool = vin_cm.__enter__()

            def emit_v_eighth(j):
                """Project V for k-rows [j*256, (j+1)*256), all 16 heads."""
                k0 = j * 256
                vin_t = vin_pool.tile([P, ET, 256], BF16, tag="vin")
                for ke in range(ET):
                    nc.sync.dma_start(
                        out=vin_t[:, ke, :],
                        in_=vinT_d[ke * P : (ke + 1) * P, k0 : k0 + 256],
                    )
                for kt_ in range(2):  # two 128-k-tiles per eighth
                    ktg = 2 * j + kt_
                    for ch in range(2):  # 512 e-cols = 8 heads each
                        ps = proj_psum.tile([P, 512], F32, tag="proj")
                        for ke in range(ET):
                            nc.tensor.matmul(
                                ps[:],
                                lhsT=vin_t[:, ke, kt_ * P : (kt_ + 1) * P],
                                rhs=wv_sb[:, ke, ch * 512 : (ch + 1) * 512],
                                start=(ke == 0),
                                stop=(ke == ET - 1),
                            )
                        nc.vector.tensor_copy(
                            v_sb[:, ktg, ch * 8 : (ch + 1) * 8, :], ps[:]
                        )

            # ---- attention: exp-leads software pipeline --------------------
            # Beat i emits: exp(beat i), S(beat i+1), PV/den(beat i-1).  The
            # scores for the NEXT beat are always issued before any filler
            # (PV/den/projection chunks), so ACT runs exp back-to-back.
            pipe = {"s": None, "exp": None, "ctx": None, "den": None}

            def emit_S(p, qc, kt_):
                s = s_psum.tile([P, 2, 512], F32, tag="s", name="s_ps")
                for h in range(2):
                    nc.tensor.matmul(
                        s[:, h, :],
                        lhsT=kT[p][h * D : (h + 1) * D, kt_ * P : (kt_ + 1) * P],
                        rhs=qT[p][h * D : (h + 1) * D, qc * 512 : (qc + 1) * 512],
                        start=True,
                        stop=True,
                    )
                pipe["s"] = (s, p, qc, kt_)

            def emit_exp():
                s, p, qc, kt_ = pipe["s"]
                exp_t = exp_pool.tile([P, 2, 512], BF16, tag="exp")
                nc.scalar.activation(
                    exp_t[:], s[:], mybir.ActivationFunctionType.Exp, scale=0.125
                )
                prev = pipe["exp"]
                pipe["exp"] = (exp_t, p, qc, kt_)
                return prev

            def emit_pv_den(exp_t, p, qc, kt_):
                """PV + denominator matmuls; finalize at the last k-tile."""
                if kt_ == 0:
                    pipe["ctx"] = ctx_psum.tile([P, 512], F32, tag="ctx", name="ctx_ps")
                    pipe["den"] = den_psum.tile([P, 512], F32, tag="den", name="den_ps")
                ctx_ps, den_ps = pipe["ctx"], pipe["den"]
                for h in range(2):
                    nc.tensor.matmul(
                        ctx_ps[h * D : (h + 1) * D, :],
                        lhsT=v_sb[:, kt_, 2 * p + h, :],
                        rhs=exp_t[:, h, :],
                        start=(kt_ == 0),
                        stop=(kt_ == NKT - 1),
                    )
                for h in range(2):
                    nc.tensor.matmul(
                        den_ps[h * D : (h + 1) * D, :],
                        lhsT=ones_mat[:, 0:D],
                        rhs=exp_t[:, h, :],
                        start=(kt_ == 0),
                        stop=(kt_ == NKT - 1),
                    )
                if kt_ == NKT - 1:
                    rb = rb_pool.tile([P, 512], F32, tag="rb")
                    nc.vector.reciprocal_approx_fast(out=rb[:], in_=den_ps[:])
                    nc.vector.tensor_mul(
                        ctx_sb[:, p, qc * 512 : (qc + 1) * 512], ctx_ps[:], rb[:]
                    )

            # ---- schedule --------------------------------------------------
            # prologue: K/Q projections for pair 0 (chunk 0 first so the
            # first S-matmul unblocks as early as possible)
            emit_pk_chunk(0, 0)
            emit_pq_chunk(0, 0)
            for c in range(1, 4):
                emit_pk_chunk(0, c)
            emit_pq_chunk(0, 1)

            beats = [(p, qc, kt_) for p in range(ET) for qc in range(NQC)
                     for kt_ in range(NKT)]

            # filler emitted after the pipeline beat for each (p, qc, kt)
            fillers = {}
            for j in range(1, 8):  # eighth 0 goes before the beat loop
                fillers.setdefault((0, 0, 2 * j - 2), []).append(
                    lambda j=j: emit_v_eighth(j)
                )
            for i_, kt_ in enumerate((10, 12, 14)):
                fillers.setdefault((0, 0, kt_), []).append(
                    lambda c=i_: emit_pk_chunk(1, c)
                )
            for p in range(ET):
                for qc in range(NQC):
                    if p == 0 and qc == 0:
                        continue
                    if p + 1 < ET:
                        chunks = (
                            [(emit_pk_chunk, p + 1, c) for c in range(3)]
                            if qc == 0
                            else [
                                (emit_pk_chunk, p + 1, 3),
                                (emit_pq_chunk, p + 1, 0),
                                (emit_pq_chunk, p + 1, 1),
                            ]
                        )
                        for i_, (fn, a0, a1) in enumerate(chunks):
                            fillers.setdefault((p, qc, 2 + 5 * i_), []).append(
                                lambda fn=fn, a0=a0, a1=a1: fn(a0, a1)
                            )
            def _wo_prefetch():
                for ke in range(ET):
                    nc.sync.dma_start(
                        out=wo_sb[:, ke, :], in_=Wo[ke * P : (ke + 1) * P, :]
                    )
            fillers.setdefault((6, 1, 8), []).append(_wo_prefetch)

            emit_v_eighth(0)
            emit_S(*beats[0])
            for i, beat in enumerate(beats):
                prev = emit_exp()
                if i + 1 < len(beats):
                    emit_S(*beats[i + 1])
                if prev is not None:
                    emit_pv_den(*prev)
                for fn in fillers.get(beat, ()):
                    fn()
            emit_pv_den(*pipe["exp"])

            vin_cm.__exit__(None, None, None)
            w_cm.__exit__(None, None, None)

            # ---- PO: out = ctx^T @ Wo + ones x (bv@Wo + bo) ----------------
            with tc.tile_pool(name="po_out", bufs=4) as out_pool:
                for qt in range(RQ // P):
                    for ch in range(2):
                        ps = proj_psum.tile([P, 512], F32, tag="proj")
                        for ke in range(ET):
                            nc.tensor.matmul(
                                ps[:],
                                lhsT=ctx_sb[:, ke, qt * P : (qt + 1) * P],
                                rhs=wo_sb[:, ke, ch * 512 : (ch + 1) * 512],
                                start=(ke == 0),
                                stop=False,
                            )
                        nc.tensor.matmul(
                            ps[:],
                            lhsT=ones_mat[:],
                            rhs=row_pad[:, ch * 512 : (ch + 1) * 512],
                            start=False,
                            stop=True,
                        )
                        out_t = out_pool.tile([P, 512], F32, tag="out_t")
                        if (qt + ch) % 2 == 0:
                            nc.vector.tensor_copy(out_t[:], ps[:])
                        else:
                            nc.scalar.copy(out_t[:], ps[:])
                        nc.sync.dma_start(
                            out=out[qt * P : (qt + 1) * P, ch * 512 : (ch + 1) * 512],
                            in_=out_t[:],
                        )

            wo_cm.__exit__(None, None, None)

    nc.compile()
    return nc


def _get_program():
    if "nc" not in _CACHE:
        _CACHE["nc"] = _build_program()
    return _CACHE["nc"]


def kernel(query, key, value, Wq, Wk, Wv, Wo, bq, bk, bv, bo):
    global _LAST_RESULTS
    query = np.asarray(query, dtype=np.float32)
    key = np.asarray(key, dtype=np.float32)
    value = np.asarray(value, dtype=np.float32)
    import ml_dtypes

    bf16 = ml_dtypes.bfloat16
    shared = {
        "Wq": np.ascontiguousarray(np.asarray(Wq, np.float32).astype(bf16)),
        "Wk": np.ascontiguousarray(np.asarray(Wk, np.float32).astype(bf16)),
        "Wv": np.ascontiguousarray(np.asarray(Wv, np.float32).astype(bf16)),
        "Wo": np.ascontiguousarray(np.asarray(Wo, np.float32).astype(bf16)),
        "bq": np.ascontiguousarray(np.asarray(bq, np.float32)),
        "bk": np.ascontiguousarray(np.asarray(bk, np.float32)),
        "bv": np.ascontiguousarray(np.asarray(bv, np.float32)),
        "bo": np.ascontiguousarray(np.asarray(bo, np.float32)),
        "row": np.ascontiguousarray(
            (np.asarray(bv, np.float64) @ np.asarray(Wo, np.float64)
             + np.asarray(bo, np.float64)).astype(np.float32)
        ),
    }
    in_maps = []
    for c in range(N_CORES):
        b, half = c // 2, c % 2
        in_maps.append(
            {
                "qinT": np.ascontiguousarray(
                    query[b, half * RQ : (half + 1) * RQ, :].T.astype(bf16)
                ),
                "kinT": np.ascontiguousarray(key[b].T.astype(bf16)),
                "vinT": np.ascontiguousarray(value[b].T.astype(bf16)),
                **shared,
            }
        )
    nc = _get_program()
    res = run_bass_kernel_spmd(nc, in_maps, list(range(N_CORES)))
    _LAST_RESULTS = res
    full = np.empty((B, S, E), dtype=np.float32)
    for c in range(N_CORES):
        b, half = c // 2, c % 2
        full[b, half * RQ : (half + 1) * RQ, :] = res.results[c]["out"]
    return full
